# revision 1
# baseline (speedup 1.0000x reference)
"""AttentionBlock (GroupNorm -> qkv 1x1 -> 4-head attention over 4096 tokens
-> proj 1x1 -> residual) distributed over 8 TRN2 NeuronCores.

Sharding: zero-communication query sharding. Core j handles batch b = j//2 and
query half qh = j%2 (2048 of the 4096 spatial positions). Each core:
  - loads the full x[b] (256, 4096) to compute GroupNorm + K/V over all keys
  - computes Q only for its 2048 queries
  - computes scores transposed ([keys, queries] via lhsT=k, rhs=q) so exp()
    output feeds the AV matmul directly with no transpose of the 4096^2
    attention matrix; softmax max-subtraction is skipped (scores are O(4)
    std-normal logits, far from fp32 exp overflow) and the denominator comes
    from a ones-column appended to v^T
  - projects, adds bias + residual, writes its (256, 2048) output slice

All matmuls use the full 128x128 PE array (contraction K=128, output
partitions M=128): per-head K tensors are built with the other head's 64
partition rows zeroed (so a full-partition contraction against the packed
2-head Q tile yields that head's scores), and the GroupNorm group-mask
matmuls are zero-padded to 128. Sub-128 contractions would select PE
array-tiling modes that production kernels avoid.
"""

import os as _os

import numpy as np

import concourse.bass as bass
import concourse.tile as tile
from concourse import bacc, mybir
from concourse.bass_utils import run_bass_kernel_spmd

C = 256
HW = 4096
NH = 4
D = 64  # head dim
G = 8  # groups
EPS = 1e-5
SCALE = D**-0.5
Q = HW // 2  # queries per core
NJT = HW // 128  # 32 key tiles
NIC = Q // 512  # 4 query chunks of 512

F32 = mybir.dt.float32
BF16 = mybir.dt.bfloat16

# compute precision for the attention path (matmuls + exp storage).
# "bf16" runs the PE at 4x the fp32 rate; GroupNorm stats, softmax
# normalization, psum accumulation and the residual stay fp32.
KDT = _os.environ.get("KDT", "bf16")
_BF = KDT == "bf16"
# KRT=rt: per-head K=64 QK matmuls row-tiled onto PE halves (tile rows 0/64)
# instead of zero-padded K=128 — halves QK PE time, relies on PE row tiling.
KRT = _os.environ.get("KRT", "kz") == "rt"
DT_QK = BF16 if _BF else F32  # h/q/k + QK^T matmul dtype
DT_E = BF16 if _BF else F32  # exp(scores) sbuf + AV matmul dtype
DT_V = BF16 if _BF else F32  # v^T sbuf dtype
DT_O = BF16 if _BF else F32  # oT sbuf + o sbuf + proj matmul dtype


def build(finalize=True):
    nc = bacc.Bacc("TRN2", target_bir_lowering=False, debug=False, num_devices=8)

    x = nc.declare_dram_parameter("x", [C, HW], F32, isOutput=False)
    xq = nc.declare_dram_parameter("xq", [C, Q], F32, isOutput=False)
    wn2 = nc.declare_dram_parameter("wn2", [128, 2], F32, isOutput=False)
    bn2 = nc.declare_dram_parameter("bn2", [128, 2], F32, isOutput=False)
    wq = nc.declare_dram_parameter("wq", [128, 2, C], DT_QK, isOutput=False)
    bq2 = nc.declare_dram_parameter("bq2", [128, 2], F32, isOutput=False)
    if KRT:
        wk = nc.declare_dram_parameter("wk", [128, 2, C], DT_QK, isOutput=False)
        bk2 = nc.declare_dram_parameter("bk2", [128, 2], F32, isOutput=False)
    else:
        wkz = nc.declare_dram_parameter("wkz", [128, 2, NH, 128], DT_QK, isOutput=False)
        bkz = nc.declare_dram_parameter("bkz", [128, NH], F32, isOutput=False)
    wv = nc.declare_dram_parameter("wv", [128, 2, NH * 65], DT_V, isOutput=False)
    vb = nc.declare_dram_parameter("vb", [128, NH * 65], F32, isOutput=False)
    wproj = nc.declare_dram_parameter("wproj", [128, 2, C], DT_O, isOutput=False)
    bproj2 = nc.declare_dram_parameter("bproj2", [128, 2], F32, isOutput=False)
    gmask = nc.declare_dram_parameter("gmask", [128, 2, 128], F32, isOutput=False)
    gmaskT = nc.declare_dram_parameter("gmaskT", [128, 2, 128], F32, isOutput=False)
    ident = nc.declare_dram_parameter("ident", [128, 128], DT_O, isOutput=False)
    out = nc.declare_dram_parameter("out", [C, Q], F32, isOutput=True)

    Exp = mybir.ActivationFunctionType.Exp
    Ln = mybir.ActivationFunctionType.Ln
    Alu = mybir.AluOpType

    with tile.TileContext(nc) as tc:
        with (
            tc.tile_pool(name="keep", bufs=1) as keep,
            tc.tile_pool(name="consts", bufs=1) as consts,
            tc.tile_pool(name="small", bufs=4) as small,
        ):
            # ---- constants ----
            WQ = consts.tile([128, 2, C], DT_QK)
            nc.sync.dma_start(out=WQ, in_=wq[:])
            if KRT:
                WK = consts.tile([128, 2, C], DT_QK)
                nc.sync.dma_start(out=WK, in_=wk[:])
                BK = consts.tile([128, 2], F32)
                nc.sync.dma_start(out=BK, in_=bk2[:])
            else:
                WKZ = consts.tile([128, 2, NH, 128], DT_QK)
                nc.sync.dma_start(out=WKZ, in_=wkz[:])
                BKZ = consts.tile([128, NH], F32)
                nc.sync.dma_start(out=BKZ, in_=bkz[:])
            WV = consts.tile([128, 2, NH * 65], DT_V)
            nc.sync.dma_start(out=WV, in_=wv[:])
            WP = consts.tile([128, 2, C], DT_O)
            nc.sync.dma_start(out=WP, in_=wproj[:])
            WN = consts.tile([128, 2], F32)
            nc.sync.dma_start(out=WN, in_=wn2[:])
            BN = consts.tile([128, 2], F32)
            nc.sync.dma_start(out=BN, in_=bn2[:])
            BQ = consts.tile([128, 2], F32)
            nc.sync.dma_start(out=BQ, in_=bq2[:])
            VB = consts.tile([128, NH * 65], F32)
            nc.sync.dma_start(out=VB, in_=vb[:])
            BP = consts.tile([128, 2], F32)
            nc.sync.dma_start(out=BP, in_=bproj2[:])
            GM = consts.tile([128, 2, 128], F32)
            nc.sync.dma_start(out=GM, in_=gmask[:])
            GMT = consts.tile([128, 2, 128], F32)
            nc.sync.dma_start(out=GMT, in_=gmaskT[:])
            IDENT = consts.tile([128, 128], DT_O)
            nc.sync.dma_start(out=IDENT, in_=ident[:])
            EPS8 = consts.tile([G, 1], F32)
            nc.vector.memset(EPS8, EPS)

            XQ = [
                keep.tile([128, Q], F32, tag=f"XQ{t}", name=f"XQ{t}")
                for t in range(2)
            ]
            for t in range(2):
                nc.sync.dma_start(out=XQ[t], in_=xq[t * 128 : (t + 1) * 128, :])

            # Per-head K with the other head's partition rows zeroed; Q keeps
            # channels on partitions (tile t holds heads 2t, 2t+1).
            if KRT:
                KZ = [
                    keep.tile([128, HW], DT_QK, tag=f"K{t}", name=f"K{t}")
                    for t in range(2)
                ]
            else:
                KZ = [
                    keep.tile([128, HW], DT_QK, tag=f"KZ{h}", name=f"KZ{h}")
                    for h in range(NH)
                ]
            QT = [
                keep.tile([128, Q], DT_QK, tag=f"Q{t}", name=f"Q{t}")
                for t in range(2)
            ]
            # V^T with a ones column per head: [keys, (head, d+1)]
            V = keep.tile([128, NJT, NH * 65], DT_V)

            with tc.tile_pool(name="xh", bufs=1) as xh:
                X = [
                    xh.tile([128, HW], F32, tag=f"X{t}", name=f"X{t}")
                    for t in range(2)
                ]
                for t in range(2):
                    nc.sync.dma_start(out=X[t], in_=x[t * 128 : (t + 1) * 128, :])
                HQ = [
                    xh.tile([128, Q], DT_QK, tag=f"HQ{t}", name=f"HQ{t}")
                    for t in range(2)
                ]

                # ---- GroupNorm statistics ----
                # per-channel mean/var over HW via bn_stats, then aggregate the
                # 32 channels of each group with a zero-padded (1/32)-mask
                # matmul (full K=128/M=128).
                with tc.tile_pool(name="stats_ps", bufs=2, space="PSUM") as sps:
                    mv2 = small.tile([128, 2, 2], F32)  # [:, t, (mean, E[x^2])]
                    for t in range(2):
                        st = small.tile([128, 8, 6], F32, tag="bnst")
                        xr = X[t].rearrange("p (n f) -> p n f", f=512)
                        for s in range(8):
                            nc.vector.bn_stats(out=st[:, s], in_=xr[:, s])
                        mv = small.tile([128, 2], F32, tag="bnmv")
                        nc.vector.bn_aggr(out=mv, in_=st)
                        # mv2 col0 = mean, col1 = var + mean^2
                        nc.vector.tensor_copy(out=mv2[:, t, 0:1], in_=mv[:, 0:1])
                        nc.vector.tensor_tensor(
                            out=mv2[:, t, 1:2], in0=mv[:, 0:1], in1=mv[:, 0:1],
                            op=Alu.mult,
                        )
                        nc.vector.tensor_tensor(
                            out=mv2[:, t, 1:2], in0=mv2[:, t, 1:2], in1=mv[:, 1:2],
                            op=Alu.add,
                        )

                    gps = sps.tile([128, 2], F32)
                    for t in range(2):
                        nc.tensor.matmul(
                            out=gps, lhsT=GM[:, t], rhs=mv2[:, t],
                            start=(t == 0), stop=(t == 1),
                        )
                    gsb = small.tile([128, 2], F32)
                    nc.vector.tensor_copy(out=gsb, in_=gps)
                    # gstat rows 0..8: col0 = group mean, col1 = rsqrt(var+eps);
                    # rows 8..128 stay zero for the padded broadcast matmul.
                    gstat = small.tile([128, 2], F32)
                    nc.vector.memset(gstat, 0.0)
                    nc.vector.tensor_copy(out=gstat[:G, 0:1], in_=gsb[:G, 0:1])
                    gvar = small.tile([G, 1], F32)
                    nc.vector.tensor_tensor(
                        out=gvar, in0=gsb[:G, 0:1], in1=gsb[:G, 0:1], op=Alu.mult
                    )
                    nc.vector.tensor_tensor(
                        out=gvar, in0=gsb[:G, 1:2], in1=gvar, op=Alu.subtract
                    )
                    # rsqrt via exp(-0.5*ln(v+eps)): stays in the ln/exp table set
                    nc.scalar.activation(out=gvar, in_=gvar, func=Ln, bias=EPS8)
                    nc.scalar.activation(
                        out=gstat[:G, 1:2], in_=gvar, func=Exp, scale=-0.5
                    )

                    # broadcast group stats back to channels
                    AB = []  # [t] -> [128, 2] (alpha, beta)
                    for t in range(2):
                        bc = sps.tile([128, 2], F32, tag="bcst", name="bcst")
                        nc.tensor.matmul(out=bc, lhsT=GMT[:, t], rhs=gstat)
                        bsb = small.tile([128, 2], F32, tag="bsb", name="bsb")
                        nc.vector.tensor_copy(out=bsb, in_=bc)
                        ab = small.tile([128, 2], F32, tag=f"ab{t}", name=f"ab{t}")
                        # alpha = rstd * w
                        nc.vector.tensor_tensor(
                            out=ab[:, 0:1], in0=bsb[:, 1:2], in1=WN[:, t : t + 1],
                            op=Alu.mult,
                        )
                        # beta = b - mean * alpha
                        nc.vector.tensor_tensor(
                            out=ab[:, 1:2], in0=bsb[:, 0:1], in1=ab[:, 0:1],
                            op=Alu.mult,
                        )
                        nc.vector.tensor_tensor(
                            out=ab[:, 1:2], in0=BN[:, t : t + 1], in1=ab[:, 1:2],
                            op=Alu.subtract,
                        )
                        AB.append(ab)

                # ---- normalized activations ----
                if DT_QK == F32:
                    H = X  # overwrite in place
                else:
                    H = [
                        xh.tile([128, HW], DT_QK, tag=f"H{t}", name=f"H{t}")
                        for t in range(2)
                    ]
                for t in range(2):
                    nc.vector.tensor_scalar(
                        out=H[t], in0=X[t],
                        scalar1=AB[t][:, 0:1], scalar2=AB[t][:, 1:2],
                        op0=Alu.mult, op1=Alu.add,
                    )
                    nc.vector.tensor_scalar(
                        out=HQ[t], in0=XQ[t],
                        scalar1=AB[t][:, 0:1], scalar2=AB[t][:, 1:2],
                        op0=Alu.mult, op1=Alu.add,
                    )

                # ---- q, k, v ----
                with (
                    tc.tile_pool(name="qkv_ps", bufs=3, space="PSUM") as qkv_ps,
                    tc.tile_pool(name="v_ps", bufs=3, space="PSUM") as v_ps,
                ):
                    if KRT:
                        for t in range(2):
                            for n in range(HW // 512):
                                ps = qkv_ps.tile(
                                    [128, 512], F32, tag="qkvps", name="kps"
                                )
                                for ct in range(2):
                                    nc.tensor.matmul(
                                        out=ps,
                                        lhsT=WK[:, ct, t * 128 : (t + 1) * 128],
                                        rhs=H[ct][:, n * 512 : (n + 1) * 512],
                                        start=(ct == 0), stop=(ct == 1),
                                    )
                                nc.vector.tensor_scalar_add(
                                    out=KZ[t][:, n * 512 : (n + 1) * 512],
                                    in0=ps, scalar1=BK[:, t : t + 1],
                                )
                    else:
                        for h in range(NH):
                            for n in range(HW // 512):
                                ps = qkv_ps.tile(
                                    [128, 512], F32, tag="qkvps", name="kps"
                                )
                                for ct in range(2):
                                    nc.tensor.matmul(
                                        out=ps,
                                        lhsT=WKZ[:, ct, h],
                                        rhs=H[ct][:, n * 512 : (n + 1) * 512],
                                        start=(ct == 0), stop=(ct == 1),
                                    )
                                nc.vector.tensor_scalar_add(
                                    out=KZ[h][:, n * 512 : (n + 1) * 512],
                                    in0=ps, scalar1=BKZ[:, h : h + 1],
                                )
                    for t in range(2):
                        for n in range(Q // 512):
                            ps = qkv_ps.tile([128, 512], F32, tag="qkvps", name="qps")
                            for ct in range(2):
                                nc.tensor.matmul(
                                    out=ps,
                                    lhsT=WQ[:, ct, t * 128 : (t + 1) * 128],
                                    rhs=HQ[ct][:, n * 512 : (n + 1) * 512],
                                    start=(ct == 0), stop=(ct == 1),
                                )
                            nc.vector.tensor_scalar_add(
                                out=QT[t][:, n * 512 : (n + 1) * 512],
                                in0=ps, scalar1=BQ[:, t : t + 1],
                            )

                    for j in range(NJT):
                        ps = v_ps.tile([128, NH * 65], F32, tag="vps", name="vps")
                        for ct in range(2):
                            nc.tensor.matmul(
                                out=ps,
                                lhsT=H[ct][:, j * 128 : (j + 1) * 128],
                                rhs=WV[:, ct],
                                start=(ct == 0), stop=(ct == 1),
                            )
                        nc.vector.tensor_tensor(
                            out=V[:, j], in0=ps, in1=VB, op=Alu.add
                        )
                        nc.vector.memset(
                            V[:, j].rearrange("p (h c) -> p h c", c=65)[:, :, 64:65],
                            1.0,
                        )

            # ---- attention + projection ----
            with (
                tc.tile_pool(name="s_ps", bufs=2, space="PSUM") as s_ps,
                tc.tile_pool(name="ot_ps", bufs=1, space="PSUM") as ot_ps,
                tc.tile_pool(name="tr_ps", bufs=1, space="PSUM") as tr_ps,
                tc.tile_pool(name="pj_ps", bufs=1, space="PSUM") as pj_ps,
                tc.tile_pool(name="att", bufs=2) as att,
                tc.tile_pool(name="exps", bufs=3) as expp,
            ):
                for ic in range(NIC):
                    # oTn[isub]: [128 queries, 256 channels] normalized attn out
                    oTn = [
                        att.tile([128, C], DT_O, tag=f"oTn{isub}", name=f"oTn{isub}")
                        for isub in range(4)
                    ]
                    for hp in range(2):
                        oT = [
                            ot_ps.tile([128, 4, 68], F32, tag=f"ot{h2}", name=f"ot{h2}")
                            for h2 in range(2)
                        ]
                        def qk_into(S, j):
                            for h2 in range(2):
                                if KRT:
                                    nc.tensor.matmul(
                                        out=S[:, h2 * 512 : (h2 + 1) * 512],
                                        lhsT=KZ[hp][
                                            h2 * 64 : (h2 + 1) * 64,
                                            j * 128 : (j + 1) * 128,
                                        ],
                                        rhs=QT[hp][
                                            h2 * 64 : (h2 + 1) * 64,
                                            ic * 512 : (ic + 1) * 512,
                                        ],
                                        start=True, stop=True,
                                    )
                                else:
                                    nc.tensor.matmul(
                                        out=S[:, h2 * 512 : (h2 + 1) * 512],
                                        lhsT=KZ[2 * hp + h2][
                                            :, j * 128 : (j + 1) * 128
                                        ],
                                        rhs=QT[hp][:, ic * 512 : (ic + 1) * 512],
                                        start=True, stop=True,
                                    )

                        def av_from(E, j):
                            for h2 in range(2):
                                head = 2 * hp + h2
                                for isub in range(4):
                                    # one psum accumulation group per oT bank:
                                    # only the first matmul starts (zeroing the
                                    # bank) and only the last stops
                                    nc.tensor.matmul(
                                        out=oT[h2][:, isub, 0:65],
                                        lhsT=E[
                                            :,
                                            h2 * 512 + isub * 128 : h2 * 512
                                            + (isub + 1) * 128,
                                        ],
                                        rhs=V[:, j, head * 65 : (head + 1) * 65],
                                        start=(j == 0 and isub == 0),
                                        stop=(j == NJT - 1 and isub == 3),
                                    )

                        # paired j iterations: both QK (row-tiled) matmul pairs
                        # issue back to back, halving PE tiling-mode switches
                        for jp in range(NJT // 2):
                            j0, j1 = 2 * jp, 2 * jp + 1
                            S0 = s_ps.tile([128, 1024], F32, tag="scores", name="s0")
                            qk_into(S0, j0)
                            S1 = s_ps.tile([128, 1024], F32, tag="scores", name="s1")
                            qk_into(S1, j1)
                            E0 = expp.tile([128, 1024], DT_E, tag="exps", name="e0")
                            nc.scalar.activation(out=E0, in_=S0, func=Exp, scale=SCALE)
                            E1 = expp.tile([128, 1024], DT_E, tag="exps", name="e1")
                            nc.scalar.activation(out=E1, in_=S1, func=Exp, scale=SCALE)
                            av_from(E0, j0)
                            av_from(E1, j1)
                        # normalize by the ones-column sums
                        for h2 in range(2):
                            head = 2 * hp + h2
                            for isub in range(4):
                                r = small.tile([128, 1], F32, tag="recip", name="recip")
                                nc.vector.reciprocal(
                                    out=r, in_=oT[h2][:, isub, 64:65]
                                )
                                nc.vector.tensor_scalar_mul(
                                    out=oTn[isub][:, head * 64 : (head + 1) * 64],
                                    in0=oT[h2][:, isub, 0:64],
                                    scalar1=r,
                                )
                    # transpose oTn -> o [channels, 512 queries]
                    OSB = [
                        att.tile([128, 512], DT_O, tag=f"osb{ct}", name=f"osb{ct}")
                        for ct in range(2)
                    ]
                    for ct in range(2):
                        for isub in range(4):
                            tp = tr_ps.tile([128, 128], DT_O, tag="trps", name="trps")
                            nc.tensor.transpose(
                                tp, oTn[isub][:, ct * 128 : (ct + 1) * 128], IDENT
                            )
                            nc.vector.tensor_copy(
                                out=OSB[ct][:, isub * 128 : (isub + 1) * 128], in_=tp
                            )
                    # proj + bias + residual
                    for mt in range(2):
                        ps = pj_ps.tile([128, 512], F32, tag="pjps", name="pjps")
                        for ct in range(2):
                            nc.tensor.matmul(
                                out=ps,
                                lhsT=WP[:, ct, mt * 128 : (mt + 1) * 128],
                                rhs=OSB[ct],
                                start=(ct == 0), stop=(ct == 1),
                            )
                        ob = att.tile([128, 512], F32, tag="outsb", name="outsb")
                        nc.vector.tensor_scalar_add(
                            out=ob, in0=ps, scalar1=BP[:, mt : mt + 1]
                        )
                        nc.vector.tensor_tensor(
                            out=ob, in0=ob,
                            in1=XQ[mt][:, ic * 512 : (ic + 1) * 512],
                            op=Alu.add,
                        )
                        nc.sync.dma_start(
                            out=out[
                                mt * 128 : (mt + 1) * 128,
                                ic * 512 : (ic + 1) * 512,
                            ],
                            in_=ob,
                        )
    if finalize:
        nc.finalize()
    return nc


def _prep_weights(norm_w, norm_b, qkv_w, qkv_b, proj_w, proj_b):
    """Host-side layout (pure reshapes/transposes + dtype casts of weights)."""
    import ml_dtypes

    f = np.float32
    cdt = ml_dtypes.bfloat16 if _BF else np.float32

    def ctile(v):  # (256,) -> (128, 2) per channel-tile columns
        return np.ascontiguousarray(np.asarray(v).reshape(2, 128).T, dtype=f)

    def ptile(m):  # (256, N) -> (128, 2, N)
        return np.ascontiguousarray(
            np.asarray(m).reshape(2, 128, -1).transpose(1, 0, 2), dtype=f
        )

    qkv_w = np.asarray(qkv_w)
    qkv_b = np.asarray(qkv_b)
    wqT = qkv_w[:C].T  # (256, 256)
    wkT = qkv_w[C : 2 * C].T  # (256, 256) key rows
    # per-head K weights, zero-padded so each head's output occupies the same
    # 64 partition rows as its q in the packed 2-head Q tile
    wkzT = np.zeros((C, NH, 128), dtype=f)
    bkz = np.zeros((128, NH), dtype=f)
    for h in range(NH):
        off = 64 * (h % 2)
        wkzT[:, h, off : off + 64] = wkT[:, h * 64 : (h + 1) * 64]
        bkz[off : off + 64, h] = qkv_b[C + h * 64 : C + (h + 1) * 64]
    kw_extra = (
        dict(
            wk=ptile(wkT).astype(cdt),
            bk2=np.ascontiguousarray(
                qkv_b[C : 2 * C].reshape(2, 128).T, dtype=f
            ),
        )
        if KRT
        else dict(
            wkz=ptile(wkzT.reshape(C, NH * 128))
            .reshape(128, 2, NH, 128)
            .astype(cdt),
            bkz=bkz,
        )
    )
    wvm = qkv_w[2 * C :]  # (256, 256)
    wvT = np.zeros((C, NH * 65), dtype=f)
    vb = np.zeros((128, NH * 65), dtype=f)
    for h in range(NH):
        wvT[:, h * 65 : h * 65 + 64] = wvm[h * 64 : (h + 1) * 64].T
        vb[:, h * 65 : h * 65 + 64] = qkv_b[2 * C + h * 64 : 2 * C + (h + 1) * 64][
            None, :
        ]
    # zero-padded group masks (value 1/32 for group-mean aggregation; one-hot
    # transpose for the broadcast back to channels)
    gm = np.zeros((C, 128), dtype=f)
    gmT = np.zeros((C, 128), dtype=f)  # row c, col g layout transposed below
    for c in range(C):
        gm[c, c // 32] = 1.0 / 32.0
        gmT[c, c // 32] = 1.0
    # gmaskT param layout [p, t, 128]: partition p = group index (only 0..8
    # nonzero), free = channel within tile t
    gmaskT = np.zeros((128, 2, 128), dtype=f)
    for c in range(C):
        gmaskT[c // 32, c // 128, c % 128] = 1.0
    return dict(
        wn2=ctile(norm_w),
        bn2=ctile(norm_b),
        wq=ptile(wqT).astype(cdt),
        bq2=np.ascontiguousarray(qkv_b[:C].reshape(2, 128).T, dtype=f),
        **kw_extra,
        wv=ptile(wvT).astype(cdt),
        vb=vb,
        wproj=ptile(np.asarray(proj_w).T).astype(cdt),
        bproj2=ctile(proj_b),
        gmask=ptile(gm),
        gmaskT=gmaskT,
        ident=np.eye(128, dtype=cdt),
    )


_NC_CACHE = {}
_RUN_OPTS = {}  # extra kwargs for run_bass_kernel_spmd (test harness sets trace)
LAST_RESULT = None


def _get_nc():
    if "nc" not in _NC_CACHE:
        _NC_CACHE["nc"] = build()
    return _NC_CACHE["nc"]


def kernel(x, norm_w, norm_b, qkv_w, qkv_b, proj_w, proj_b, **_):
    nc = _get_nc()
    w = _prep_weights(norm_w, norm_b, qkv_w, qkv_b, proj_w, proj_b)
    x = np.asarray(x, dtype=np.float32)
    Bv, Cv, Hv, Wv = x.shape
    xf = x.reshape(Bv, Cv, Hv * Wv)
    in_maps = []
    for j in range(8):
        b, qh = j // 2, j % 2
        m = dict(w)
        m["x"] = np.ascontiguousarray(xf[b])
        m["xq"] = np.ascontiguousarray(xf[b][:, qh * Q : (qh + 1) * Q])
        in_maps.append(m)
    res = run_bass_kernel_spmd(nc, in_maps, core_ids=list(range(8)), **_RUN_OPTS)
    global LAST_RESULT
    LAST_RESULT = res
    outf = np.empty((Bv, Cv, Hv * Wv), dtype=np.float32)
    for j in range(8):
        b, qh = j // 2, j % 2
        outf[b][:, qh * Q : (qh + 1) * Q] = res.results[j]["out"]
    return outf.reshape(Bv, Cv, Hv, Wv)

